# revision 14
# baseline (speedup 1.0000x reference)
"""CRF NLL (forward-algorithm partition function) on 8 Trainium2 NeuronCores.

Math: the reference computes  mean_b( logZ[b] - score[b] )  where logZ is a
logsumexp forward recursion over S-1 = 2047 steps with transition matrix
T [L,L] and emissions [B,S,L]; score is a pure gather path (host).

Device strategy — "warmup telescoping" (batch sharded 8 ways, 16 seq/core):
  The linear-space recursion p_{t+1} = diag(e_t) A p_t (A = expT^T) is a
  product of strongly contractive positive maps: the projective direction
  of any chain forgets its initial condition at ~e^-1.7/step (measured
  s2/s1 of an 8-step transfer product is ~1.5e-6, 32-step ~1e-16).
  So split the 2047 steps into C segments and run C INDEPENDENT forward
  chains in lockstep, chain c starting W~8 steps before its segment from
  an arbitrary positive vector (warmup). After warmup the chain state is
  proportional to the true forward vector, and logZ telescopes through
  colsum ratios:
      logZ = sum_c log sum(u_c) - sum_{c>=2} log sum(v_c) + c0 + D*nsteps
  where v_c = chain state at segment start (after warmup), u_c = state at
  segment end (endv folded into the last emission of the last chain), and
  chain 1 is started EXACTLY from p0 (reset at its warmup boundary).
  Warmup also absorbs ragged segment lengths (2047 = 23*89 has no nice
  divisors): shorter segments simply get one extra warmup step.

  Per slot the device does G=2 matmuls [128 x 512] (stationary expT, bf16)
  and 2 DVE tensor_muls folding the emissions — 35 slots total (C=64, W=3)
  instead of the baseline's 1024 serially-dependent slots. Emission chunks
  stream over both HWDGE queues (SP + ACT halves per chunk); chain-state
  snapshots DMA out on the GPSIMD queue; states ship as bf16 and the
  colsums/logs happen on host in float64. Measured ~41-57 us/pass on 8
  cores (ambient-load dependent) vs 592-679 us for the serial fb2
  baseline; DVE elementwise throughput (~1ns/col) is the binding engine.
"""

import os
from contextlib import ExitStack

import numpy as np

B, S, L = 128, 2048, 128
NCORES = 8
BS = B // NCORES  # 16 sequences per core
IGNORE = -100

D_SHIFT = 5.829        # expected per-step log growth (measured offline)

# warm-telescoping config (shipped)
WARM_C = 64            # number of chains/segments per core
WARM_W = 3             # min warmup steps per chain
WARM_TC = 2            # emission slots per DMA chunk
WARM_GROUPS = 2        # matmul column groups (each <= 512 cols)
WARM_GP_COLS = 0       # trailing columns of the elementwise mul on GPSIMD

# test.py introspection
LAST_EXEC_TIME_NS = None
LAST_TRACE_PATH = None

_BUILT = {}


def _warm_geometry(nsteps, C, W):
    o = np.round(np.linspace(0, nsteps, C + 1)).astype(int)
    segs = np.diff(o)
    n_apply = int((W + segs).max())
    Wc = n_apply - segs            # per-chain warmup length
    snap_slots = sorted(set((Wc[1:] - 1).tolist()))   # v_c snapshot slots
    return o, segs, n_apply, Wc, snap_slots


def _build_warm(nsteps=S - 1, C=WARM_C, W=WARM_W, repeat=1, Tc=WARM_TC,
                groups=WARM_GROUPS, gp_cols=WARM_GP_COLS, fuse_tm=False,
                em_bufs=6):
    import concourse.bacc as bacc
    import concourse.tile as tile
    from concourse import mybir

    f32 = mybir.dt.float32
    cdt = mybir.dt.bfloat16

    o, segs, n_apply, Wc, snap_slots = _warm_geometry(nsteps, C, W)
    Wtot = C * BS
    # gp_cols > 0 carves a trailing column stripe into its own matmul group;
    # that stripe's emission fold runs as ACT copy (PSUM->SBUF bf16) + GPSIMD
    # multiply, offloading the saturated DVE.
    Wd = Wtot - gp_cols
    gb = [(g * Wd // groups, (g + 1) * Wd // groups) for g in range(groups)]
    if gp_cols > 0:
        gb.append((Wd, Wtot))
    for lo, hi in gb:
        assert hi - lo <= 512

    nc = bacc.Bacc(debug=False, name="crf_warm")
    with tile.TileContext(nc) as tc:
        with ExitStack() as ctx:
            d_expT = nc.dram_tensor("expT", [L, L], cdt, kind="ExternalInput")
            d_p0 = nc.dram_tensor("p0", [L, BS], cdt, kind="ExternalInput")
            d_E = nc.dram_tensor("emis", [L, n_apply, Wtot], cdt,
                                 kind="ExternalInput")
            d_snap = [
                nc.dram_tensor(f"v{i}", [L, Wtot], cdt, kind="ExternalOutput")
                for i in range(len(snap_slots))
            ]
            d_fin = nc.dram_tensor("fin", [L, Wtot], cdt, kind="ExternalOutput")

            const = ctx.enter_context(tc.tile_pool(name="const", bufs=1))
            empool = ctx.enter_context(tc.tile_pool(name="empool", bufs=em_bufs))
            ppool = ctx.enter_context(tc.tile_pool(name="ppool", bufs=4))
            sclp = ctx.enter_context(tc.tile_pool(name="sclp", bufs=2))
            zpsum = ctx.enter_context(
                tc.tile_pool(name="zpsum", bufs=2, space="PSUM"))

            expT_sb = const.tile([L, L], cdt)
            nc.sync.dma_start(out=expT_sb, in_=d_expT[:])
            p0_sb = const.tile([L, BS], cdt)
            nc.sync.dma_start(out=p0_sb, in_=d_p0[:])

            snap_of = {sl: i for i, sl in enumerate(snap_slots)}

            def _run_chain():
                pk = ppool.tile([L, Wtot], cdt, tag="p")
                nc.vector.memset(pk, 1.0)

                em_tile = None
                chunk_lo = -1
                for j in range(n_apply):
                    ci = j // Tc
                    if ci != chunk_lo:
                        chunk_lo = ci
                        lo = ci * Tc
                        hi = min(lo + Tc, n_apply)
                        em_tile = empool.tile([L, Tc, Wtot], cdt, tag="em")
                        # split every chunk across both HWDGE queues so the
                        # two DMA rings stream concurrently
                        mid = Wtot // 2
                        nc.sync.dma_start(out=em_tile[:, : hi - lo, :mid],
                                          in_=d_E[:, lo:hi, :mid])
                        nc.scalar.dma_start(out=em_tile[:, : hi - lo, mid:],
                                            in_=d_E[:, lo:hi, mid:])
                    em_sl = em_tile[:, j % Tc, :]

                    pk_new = ppool.tile([L, Wtot], cdt, tag="p")
                    if fuse_tm:
                        zz = zpsum.tile([L, Wtot], f32, tag="z")
                        for glo, ghi in gb:
                            nc.tensor.matmul(zz[:, glo:ghi], lhsT=expT_sb,
                                             rhs=pk[:, glo:ghi],
                                             start=True, stop=True)
                        nc.vector.tensor_mul(pk_new, zz, em_sl)
                    else:
                        for g, (glo, ghi) in enumerate(gb):
                            zz = zpsum.tile([L, ghi - glo], f32, tag=f"z{g}")
                            nc.tensor.matmul(zz, lhsT=expT_sb,
                                             rhs=pk[:, glo:ghi],
                                             start=True, stop=True)
                            stripe = gp_cols > 0 and g == len(gb) - 1
                            if stripe:
                                zc = sclp.tile([L, gp_cols], cdt, tag="zc")
                                nc.scalar.copy(out=zc, in_=zz)
                                nc.gpsimd.tensor_mul(pk_new[:, glo:ghi], zc,
                                                     em_sl[:, glo:ghi])
                            else:
                                nc.vector.tensor_mul(pk_new[:, glo:ghi], zz,
                                                     em_sl[:, glo:ghi])

                    if j == Wc[0] - 1:
                        # chain 1 starts exactly from p0 after its warmup
                        nc.scalar.copy(out=pk_new[:, 0:BS], in_=p0_sb)
                    if j in snap_of:
                        nc.gpsimd.dma_start(out=d_snap[snap_of[j]][:],
                                            in_=pk_new)
                    pk = pk_new

                nc.sync.dma_start(out=d_fin[:], in_=pk)

            if repeat == 1:
                _run_chain()
            else:
                with tc.For_i(0, repeat, 1):
                    _run_chain()

    nc.compile()
    return nc


def _prepare_in_maps_warm(emissions, transitions, start_transitions,
                          end_transitions, nsteps=S - 1, C=WARM_C, W=WARM_W):
    """Host packing for the warm-telescoping program -> (in_maps, c0)."""
    import ml_dtypes
    cdt = ml_dtypes.bfloat16

    o, segs, n_apply, Wc, snap_slots = _warm_geometry(nsteps, C, W)

    expT = np.exp(transitions, dtype=np.float32)

    alpha0 = start_transitions[None, :] + emissions[:, 0, :]      # [B, L]
    c0 = alpha0.max(axis=1)
    p0_all = np.exp(alpha0 - c0[:, None]).T.astype(np.float32)    # [l, B]
    endv = np.exp(end_transitions, dtype=np.float32)              # [L]

    # emission index consumed by chain c at slot j: o[c] - Wc[c] + 1 + j
    # (1-based steps; Et index = that - 1)
    it = (o[:-1] - Wc)[:, None] + np.arange(n_apply)[None, :]     # [C, n_apply]
    it0 = int(Wc[0])

    in_maps = []
    for kc in range(NCORES):
        bs = slice(kc * BS, (kc + 1) * BS)
        Et = np.exp(
            np.ascontiguousarray(
                np.transpose(emissions[bs, 1: nsteps + 1, :], (2, 1, 0))
            )
            - np.float32(D_SHIFT),
            dtype=np.float32,
        )                                                         # [l, n, b]
        Ep = Et[:, it, :]                                         # [l,C,n_apply,b]
        Ep[:, 0, :it0, :] = np.float32(np.exp(-D_SHIFT))          # chain-1 filler
        Ep[:, C - 1, n_apply - 1, :] *= endv[:, None]             # fold endv
        Ep = np.ascontiguousarray(
            np.transpose(Ep, (0, 2, 1, 3)).reshape(L, n_apply, C * BS)
        )
        in_maps.append(
            {
                "expT": expT.astype(cdt),
                "p0": np.ascontiguousarray(p0_all[:, bs]).astype(cdt),
                "emis": Ep.astype(cdt),
            }
        )
    return in_maps, c0


def _combine_warm(results, c0, nsteps=S - 1, C=WARM_C, W=WARM_W):
    """Host combine: f64 colsums + telescoped logs -> logZ [B]."""
    o, segs, n_apply, Wc, snap_slots = _warm_geometry(nsteps, C, W)
    snap_of = {sl: i for i, sl in enumerate(snap_slots)}
    logz = np.empty(B, dtype=np.float64)
    for kc in range(NCORES):
        res = results[kc]
        fin = np.asarray(res["fin"], dtype=np.float64).reshape(L, C, BS)
        vs = [np.asarray(res[f"v{i}"], dtype=np.float64).reshape(L, C, BS)
              for i in range(len(snap_slots))]
        R2 = fin.sum(axis=0)                                      # [C, BS]
        lz = np.log(R2).sum(axis=0)
        for c in range(1, C):
            v = vs[snap_of[Wc[c] - 1]]
            lz -= np.log(v[:, c, :].sum(axis=0))
        logz[kc * BS: (kc + 1) * BS] = lz
    return logz + c0.astype(np.float64) + D_SHIFT * nsteps


def _forward_device_warm(emissions, transitions, start_transitions,
                         end_transitions, nsteps=S - 1):
    from concourse.bass_utils import run_bass_kernel_spmd

    global LAST_EXEC_TIME_NS, LAST_TRACE_PATH

    in_maps, c0 = _prepare_in_maps_warm(
        emissions, transitions, start_transitions, end_transitions, nsteps
    )
    key = ("warm", nsteps, WARM_C, WARM_W)
    if key not in _BUILT:
        _BUILT[key] = _build_warm(nsteps, WARM_C, WARM_W)
    nc = _BUILT[key]
    trace = os.environ.get("CRF_TRACE", "") == "1"
    res = run_bass_kernel_spmd(
        nc, in_maps, core_ids=list(range(NCORES)), trace=trace
    )
    LAST_EXEC_TIME_NS = res.exec_time_ns
    if res.instructions_and_trace is not None:
        LAST_TRACE_PATH = res.instructions_and_trace[1]
    return _combine_warm(res.results, c0, nsteps)


def _score_host(emissions, mask, tags, transitions, start_transitions,
                end_transitions):
    """Gold path score, matching reference._crf_nll's gather path. float64."""
    em = emissions.astype(np.float64)
    T = transitions.astype(np.float64)
    startT = start_transitions.astype(np.float64)
    endT = end_transitions.astype(np.float64)

    valid = tags != IGNORE
    tags_safe = np.where(valid, tags, 0).astype(np.int64)
    vf = valid.astype(np.float64)

    score = startT[tags_safe[:, 0]] * vf[:, 0]
    prev_t = tags_safe[:, :-1]
    curr_t = tags_safe[:, 1:]
    trans_sc = T[prev_t, curr_t]
    em_sc = np.take_along_axis(em[:, 1:, :], curr_t[:, :, None], axis=2)[..., 0]
    score = score + np.sum((trans_sc + em_sc) * vf[:, 1:], axis=1)

    pos = np.arange(tags.shape[1])
    last_idx = np.max(np.where(valid, pos[None, :], -1), axis=1)
    last_tag = tags_safe[np.arange(tags.shape[0]), np.clip(last_idx, 0, S - 1)]
    score = score + np.where(last_idx >= 0, endT[last_tag], 0.0)
    return score


def _forward_numpy(emissions, mask, transitions, start_transitions,
                   end_transitions):
    """Fallback exact forward recursion (used only if mask isn't all ones)."""
    em = emissions.astype(np.float64)
    T = transitions.astype(np.float64)
    alpha = start_transitions.astype(np.float64)[None, :] + em[:, 0, :]
    for t in range(1, em.shape[1]):
        m = alpha.max(axis=1, keepdims=True)
        new = m + np.log(np.exp(alpha - m) @ np.exp(T)) + em[:, t, :]
        alpha = np.where(mask[:, t][:, None], new, alpha)
    m = alpha.max(axis=1, keepdims=True)
    return (
        m[:, 0]
        + np.log(
            np.exp(alpha - m) @ np.exp(end_transitions.astype(np.float64))
        )
    )


def kernel(emissions, mask, tags, transitions, start_transitions,
           end_transitions):
    emissions = np.asarray(emissions, dtype=np.float32)
    mask = np.asarray(mask)
    tags = np.asarray(tags)
    transitions = np.asarray(transitions, dtype=np.float32)
    start_transitions = np.asarray(start_transitions, dtype=np.float32)
    end_transitions = np.asarray(end_transitions, dtype=np.float32)

    if bool(mask.all()):
        logz = _forward_device_warm(
            emissions, transitions, start_transitions, end_transitions
        )
    else:
        logz = _forward_numpy(
            emissions, mask, transitions, start_transitions, end_transitions
        )

    score = _score_host(
        emissions, mask, tags, transitions, start_transitions, end_transitions
    )
    return np.asarray(np.mean(logz - score), dtype=np.float32)
